# revision 1
# baseline (speedup 1.0000x reference)
"""Causal multi-head attention (B=2, L=2048, D=2048, H=32) on 8 trn2 NeuronCores.

Sharding: data-parallel over batch (2 groups of 4 cores) x tensor-parallel over
heads (8 heads per core). Each core computes, for its batch b and head range:
  qhT/khT = (W [dh,D]) @ x.T  (head dims on partitions, tokens on free axis)
  vh      = x @ W.T           (tokens on partitions: natural layout)
  S.T[k,q] = khT.T-block matmuls (contraction over head dim, K=64)
  P.T = exp(S.T) with causal masking (host-precomputed 128x128 triangle mask)
  o.T[d,q] accumulated over k-chunks; softmax denominator rides along as an
  appended ones-column of V (even heads) or a separate M=1 matmul (odd heads)
  normalize via PE broadcast of 1/denom, then out.T = Wo_shard.T.T @ o.T
Host sums the 4 tensor-parallel partials per batch.

All matmuls contract over the partition axis, so the host pre-transposes
q/k/v (free on host, avoids all on-device transposes). Everything is fp32.
"""

import sys

sys.path.insert(0, "/opt/trn_rl_repo")

import numpy as np

import concourse.bass as bass
import concourse.tile as tile
from concourse import bacc, mybir
from concourse.bass_utils import run_bass_kernel_spmd


def _ensure_ntff_hook():
    """The agent image's antenv package lacks axon_hooks, which makes
    run_bass_kernel_spmd(trace=True) crash on import. Provide the module and
    register the ctypes-based NTFF profiling hook (degrades silently)."""
    try:
        import types

        import antenv

        if "antenv.axon_hooks" not in sys.modules:
            m = types.ModuleType("antenv.axon_hooks")
            state = {"hook": None}
            m.set_axon_ntff_profile_hook = lambda h: state.__setitem__("hook", h)
            m.get_axon_ntff_profile_hook = lambda: state["hook"]
            sys.modules["antenv.axon_hooks"] = m
            antenv.axon_hooks = m
        from antenv.axon_hooks import (
            get_axon_ntff_profile_hook,
            set_axon_ntff_profile_hook,
        )

        if get_axon_ntff_profile_hook() is None:
            from trn_agent_boot.trn_boot import _ntff_profile_via_ctypes

            set_axon_ntff_profile_hook(
                _ntff_profile_via_ctypes("/opt/axon/libaxon_pjrt.so")
            )
    except Exception:
        pass


_ensure_ntff_hook()

F32 = mybir.dt.float32
F32R = mybir.dt.float32r

B, L, D, H = 2, 2048, 2048, 32
HD = 64          # head dim
N_CORES = 8
TP = 4           # tensor-parallel width (heads split 4 ways)
HPC = H // TP    # heads per core = 8
DH = HPC * HD    # per-core projected width = 512
SCALE = float(HD) ** -0.5

QB = 512         # query-block width for SDPA
XT = 256         # token-tile width for the projection streaming operand


def _emit(nc, L_=L, D_=D):
    KC = D_ // 128          # contraction chunks for the projections
    NQB = L_ // QB          # query blocks
    NXT = L_ // XT          # projection token tiles
    TC = L_ // 128          # 128-token chunks
    MCH = DH // 128         # head-pair chunks = 4

    xq = nc.dram_tensor("xq", [D_, L_], F32R, kind="ExternalInput")
    xk = nc.dram_tensor("xk", [D_, L_], F32R, kind="ExternalInput")
    xv = nc.dram_tensor("xv", [D_, L_], F32R, kind="ExternalInput")
    wq = nc.dram_tensor("wq", [D_, DH], F32R, kind="ExternalInput")
    wk = nc.dram_tensor("wk", [D_, DH], F32R, kind="ExternalInput")
    wv = nc.dram_tensor("wv", [D_, DH], F32R, kind="ExternalInput")
    wo = nc.dram_tensor("wo", [DH, D_], F32R, kind="ExternalInput")
    konst = nc.dram_tensor("konst", [128, 640], F32R, kind="ExternalInput")
    outT = nc.dram_tensor("outT", [D_, L_], F32, kind="ExternalOutput")

    EXP = mybir.ActivationFunctionType.Exp

    with tile.TileContext(nc) as tc:
        from contextlib import ExitStack

        with ExitStack() as st:
            constp = st.enter_context(tc.tile_pool(name="const", bufs=1))
            ksb = constp.tile([128, 640], F32R)
            nc.sync.dma_start(ksb[:], konst[:])
            tri_sb = ksb[:, 0:128]
            ones_sb = constp.tile([128, 64], F32)
            nc.vector.memset(ones_sb[:], 1.0)

            actp = st.enter_context(tc.tile_pool(name="acts", bufs=1))
            qhT = actp.tile([128, MCH, L_], F32R)
            khT = actp.tile([128, MCH, L_], F32R)
            # vh: per 128-token chunk, 8 heads x (64 v-dims + ones col)
            vh = actp.tile([128, TC, HPC * (HD + 1)], F32R)
            # ones columns (softmax denominator trick): copy from konst block
            vh_r = vh[:, :, :].rearrange("p t (h c) -> p t h c", c=HD + 1)
            nc.vector.tensor_copy(
                vh_r[:, :, :, HD : HD + 1],
                ksb[:, 128 : 128 + TC * HPC].rearrange(
                    "p (t h one) -> p t h one", h=HPC, one=1
                ),
            )

            # ---- q/k projections: out[dim_chunk, tokens] = w_chunk.T @ xT ----
            for name, xdram, wdram, dst in (("q", xq, wq, qhT), ("k", xk, wk, khT)):
                with (
                    tc.tile_pool(name=f"w{name}", bufs=1) as wp,
                    tc.tile_pool(name=f"x{name}", bufs=2) as xp,
                    tc.tile_pool(name=f"ps{name}", bufs=4, space="PSUM") as pp,
                ):
                    w_sb = wp.tile([128, KC, DH], F32R, tag="w")
                    nc.sync.dma_start(
                        w_sb[:], wdram[:].rearrange("(kc p) m -> p kc m", p=128)
                    )
                    for n in range(NXT):
                        x_sb = xp.tile([128, KC, XT], F32R, tag="x")
                        nc.sync.dma_start(
                            x_sb[:],
                            xdram[:, n * XT : (n + 1) * XT].rearrange(
                                "(kc p) t -> p kc t", p=128
                            ),
                        )
                        for m in range(MCH):
                            ps = pp.tile([128, XT], F32, tag="ps")
                            for kc in range(KC):
                                nc.tensor.matmul(
                                    ps[:],
                                    w_sb[:, kc, m * 128 : (m + 1) * 128],
                                    x_sb[:, kc, :],
                                    start=(kc == 0),
                                    stop=(kc == KC - 1),
                                )
                            nc.vector.tensor_copy(
                                dst[:, m, n * XT : (n + 1) * XT], ps[:]
                            )

            # ---- v projection: natural layout, x chunk is the stationary op ----
            with (
                tc.tile_pool(name="wvp", bufs=1) as wp,
                tc.tile_pool(name="xvp", bufs=2) as xp,
                tc.tile_pool(name="psv", bufs=4, space="PSUM") as pp,
            ):
                w_sb = wp.tile([128, KC, DH], F32R, tag="w")
                nc.sync.dma_start(
                    w_sb[:], wv[:].rearrange("(kc p) m -> p kc m", p=128)
                )
                for n in range(NXT):
                    x_sb = xp.tile([128, KC, XT], F32R, tag="x")
                    nc.sync.dma_start(
                        x_sb[:],
                        xv[:, n * XT : (n + 1) * XT].rearrange(
                            "(kc p) t -> p kc t", p=128
                        ),
                    )
                    for tt in range(XT // 128):
                        ps = pp.tile([128, DH], F32, tag="ps")
                        for kc in range(KC):
                            nc.tensor.matmul(
                                ps[:],
                                x_sb[:, kc, tt * 128 : (tt + 1) * 128],
                                w_sb[:, kc, :],
                                start=(kc == 0),
                                stop=(kc == KC - 1),
                            )
                        tci = n * (XT // 128) + tt
                        dst = vh[:, tci, :].rearrange("p (h c) -> p h c", c=HD + 1)
                        nc.vector.tensor_copy(
                            dst[:, :, 0:HD],
                            ps[:].rearrange("p (h d) -> p h d", d=HD),
                        )

            # ---- SDPA + output accumulation ----
            otp = st.enter_context(tc.tile_pool(name="otp", bufs=1))
            oT = otp.tile([128, MCH, L_], F32R)
            with (
                tc.tile_pool(name="pp", bufs=20) as ppool,
                tc.tile_pool(name="dsbp", bufs=3) as dsbp,
                tc.tile_pool(name="stgp", bufs=3) as stgp,
                tc.tile_pool(name="sps", bufs=3, space="PSUM") as spool,
                tc.tile_pool(name="ops", bufs=3, space="PSUM") as opool,
                tc.tile_pool(name="bcps", bufs=2, space="PSUM") as bcpool,
            ):
                # Software pipeline over (head, q-block) blocks so the PE
                # never stalls on the ACT exp latency: block B's o-matmuls
                # are interleaved with block B+1's score matmuls, and the
                # normalization (which waits on a DVE reciprocal) trails by
                # two blocks.
                blocks = [(h, qb) for h in range(HPC) for qb in range(NQB)]
                KPQ = QB // 128
                state = {}

                def kcnt_of(b):
                    return (b[1] + 1) * KPQ

                def emit_s_step(b, kc):
                    h, qb = b
                    half, mch = 64 * (h % 2), h // 2
                    q0 = qb * QB
                    dj = kc - qb * KPQ
                    col0 = 128 * dj if dj > 0 else 0
                    s_ps = spool.tile([128, QB], F32, tag="s", name="s_ps")
                    nc.tensor.matmul(
                        s_ps[:, col0:QB],
                        khT[half : half + 64, mch, kc * 128 : (kc + 1) * 128],
                        qhT[half : half + 64, mch, q0 + col0 : q0 + QB],
                        start=True,
                        stop=True,
                    )
                    p_sb = ppool.tile([128, QB], F32R, tag="p", name="p_sb")
                    if col0 > 0:
                        nc.vector.tensor_copy(
                            p_sb[:, 0:col0], ksb[:, 256 : 256 + col0]
                        )
                    nc.scalar.activation(p_sb[:, col0:QB], s_ps[:, col0:QB], EXP)
                    if dj >= 0:
                        nc.vector.tensor_mul(
                            p_sb[:, col0 : col0 + 128],
                            p_sb[:, col0 : col0 + 128],
                            tri_sb[:],
                        )
                    state[b]["p"].append(p_sb)

                def emit_o_step(b, kc):
                    # One accumulation per head at psum base 0: 64 o-rows plus
                    # the denominator row from the ones-column of vh. (f32r
                    # matmuls reject a column tile_position, so odd heads
                    # can't target psum rows 64-127 directly; they stage in
                    # SBUF and DMA into oT's upper partitions.)
                    h, qb = b
                    st_ = state[b]
                    if kc == 0:
                        st_["o"] = opool.tile([128, QB], F32, tag="o", name="o_ps")
                    nc.tensor.matmul(
                        st_["o"][0:65, :],
                        vh[:, kc, h * (HD + 1) : h * (HD + 1) + HD + 1],
                        st_["p"][kc][:],
                        start=(kc == 0),
                        stop=(kc == kcnt_of(b) - 1),
                    )

                def emit_recip(b):
                    st_ = state[b]
                    dsb = dsbp.tile([65, 2 * QB], F32, tag="dsb", name="dsb")
                    nc.vector.tensor_copy(dsb[64:65, 0:QB], st_["o"][64:65, :])
                    nc.vector.reciprocal(
                        dsb[64:65, QB : 2 * QB], dsb[64:65, 0:QB]
                    )
                    st_["dsb"] = dsb

                def emit_norm(b):
                    h, qb = b
                    mch, q0 = h // 2, qb * QB
                    st_ = state.pop(b)
                    bc_ps = bcpool.tile([128, QB], F32, tag="bc", name="bc_ps")
                    nc.tensor.matmul(
                        bc_ps[0:64, :],
                        ones_sb[64:65, 0:64],
                        st_["dsb"][64:65, QB : 2 * QB],
                        start=True,
                        stop=True,
                    )
                    # At most one PSUM input per vector op: stage o into SBUF
                    # first, then scale by 1/denom.
                    if h % 2 == 0:
                        dst = oT[0:64, mch, q0 : q0 + QB]
                        nc.vector.tensor_copy(dst, st_["o"][0:64, :])
                        nc.vector.tensor_mul(dst, dst, bc_ps[0:64, :])
                    else:
                        stg = stgp.tile([64, QB], F32R, tag="stg", name="stg")
                        nc.vector.tensor_copy(stg[:], st_["o"][0:64, :])
                        nc.vector.tensor_mul(stg[:], stg[:], bc_ps[0:64, :])
                        nc.sync.dma_start(oT[64:128, mch, q0 : q0 + QB], stg[:])

                seq = blocks + [None, None]
                for idx, b in enumerate(seq):
                    prev = seq[idx - 1] if idx >= 1 else None
                    prev2 = seq[idx - 2] if idx >= 2 else None
                    if b is not None:
                        state[b] = {"p": []}
                    ns = kcnt_of(b) if b is not None else 0
                    no = kcnt_of(prev) if prev is not None else 0
                    for i in range(max(ns, no)):
                        if i < ns:
                            emit_s_step(b, i)
                        if i < no:
                            emit_o_step(prev, i)
                    if prev is not None:
                        emit_recip(prev)
                    if prev2 is not None:
                        emit_norm(prev2)

            # ---- output projection: outT[m,n] = wo_chunk.T @ oT ----
            with (
                tc.tile_pool(name="wop", bufs=1) as wop,
                tc.tile_pool(name="fps", bufs=8, space="PSUM") as fpool,
                tc.tile_pool(name="osbp", bufs=3) as osbp,
            ):
                wo_sb = wop.tile([128, MCH, D_], F32R)
                nc.sync.dma_start(
                    wo_sb[:], wo[:].rearrange("(kc p) m -> p kc m", p=128)
                )
                for m in range(D_ // 128):
                    pts = []
                    for n in range(NQB):
                        pt = fpool.tile([128, QB], F32, tag="f")
                        pts.append(pt)
                    for kc2 in range(MCH):
                        for n in range(NQB):
                            nc.tensor.matmul(
                                pts[n][:],
                                wo_sb[:, kc2, m * 128 : (m + 1) * 128],
                                oT[:, kc2, n * QB : (n + 1) * QB],
                                start=(kc2 == 0),
                                stop=(kc2 == MCH - 1),
                            )
                    for n in range(NQB):
                        osb = osbp.tile([128, QB], F32, tag="ot")
                        nc.vector.tensor_copy(osb[:], pts[n][:])
                        nc.sync.dma_start(
                            outT[m * 128 : (m + 1) * 128, n * QB : (n + 1) * QB],
                            osb[:],
                        )
    return nc


def build(L_=L, D_=D):
    nc = bacc.Bacc("TRN2", target_bir_lowering=False, debug=False)
    _emit(nc, L_, D_)
    nc.compile()
    return nc


_NC_CACHE = {}


def _get_nc():
    if "nc" not in _NC_CACHE:
        _NC_CACHE["nc"] = build()
    return _NC_CACHE["nc"]


def make_in_maps(q, k, v, Wq, Wk, Wv, Wo):
    konst_m = np.zeros((128, 640), dtype=np.float32)
    konst_m[:, 0:128] = np.triu(np.ones((128, 128), dtype=np.float32))
    konst_m[:, 128:256] = 1.0
    qT = [np.ascontiguousarray(q[b].T) for b in range(B)]
    kT = [np.ascontiguousarray(k[b].T) for b in range(B)]
    vT = [np.ascontiguousarray(v[b].T) for b in range(B)]
    wq_s, wk_s, wv_s, wo_s = [], [], [], []
    for tp in range(TP):
        rows = slice(tp * DH, (tp + 1) * DH)
        wq_s.append(np.ascontiguousarray(Wq[rows].T * SCALE))
        wk_s.append(np.ascontiguousarray(Wk[rows].T))
        wv_s.append(np.ascontiguousarray(Wv[rows].T))
        wo_s.append(np.ascontiguousarray(Wo[:, rows].T))
    in_maps = []
    for c in range(N_CORES):
        b, tp = c // TP, c % TP
        in_maps.append(
            {
                "xq": qT[b],
                "xk": kT[b],
                "xv": vT[b],
                "wq": wq_s[tp],
                "wk": wk_s[tp],
                "wv": wv_s[tp],
                "wo": wo_s[tp],
                "konst": konst_m,
            }
        )
    return in_maps


def kernel(q, k, v, Wq, Wk, Wv, Wo, mask=None, trace=False):
    q = np.asarray(q, dtype=np.float32)
    k = np.asarray(k, dtype=np.float32)
    v = np.asarray(v, dtype=np.float32)
    nc = _get_nc()
    in_maps = make_in_maps(
        q, k, v,
        np.asarray(Wq, np.float32), np.asarray(Wk, np.float32),
        np.asarray(Wv, np.float32), np.asarray(Wo, np.float32),
    )
    res = run_bass_kernel_spmd(
        nc, in_maps, core_ids=list(range(N_CORES)), trace=trace
    )
    out = np.zeros((B, L, D), dtype=np.float32)
    for c in range(N_CORES):
        out[c // TP] += res.results[c]["outT"].T
    if trace:
        return out, res
    return out



# revision 7
# speedup vs baseline: 1.6542x; 1.6542x over previous
"""Causal multi-head attention (B=2, L=2048, D=2048, H=32) on 8 trn2 NeuronCores.

Sharding: data-parallel over batch (2 groups of 4 cores) x tensor-parallel over
heads (8 heads per core). Host pre-transposes x and pre-shards/scales weights;
host sums the 4 tensor-parallel partials per batch (fp32).

v2 design (vs the f32r baseline):
  - fp16 operands everywhere (bf16 for p/vh so exp of unmasked garbage can't
    overflow 16-bit range); fp32 PSUM accumulation; fp16 output partials.
    4x cheaper LDWEIGHTS (FWL), 2x cheaper DMA, 2-4x cheaper DVE ops.
  - N=512 moving operands on every matmul (vs 256) -> half the instruction
    count on the projections.
  - S matmuls run as head PAIRS on row-tiles (0,0)/(64,0) of the PE array
    (K=64 each) -> the two matmuls execute concurrently.
  - exp batched over [128,2,512] PSUM double-tiles -> one ACT instruction per
    head-pair k-chunk.
  - PV matmuls restricted to the causally valid column range -> no zero-fill
    of p, fewer PE columns.
  - softmax denominators: rows collected at partition 64 (no partition shift),
    PE-gathered onto partitions (K=1 matmuls), ONE batched DVE reciprocal per
    head-group [128,32], then PE-broadcast back to [64,512] per block-head.
  - triangle masking on gpsimd; PSUM->SBUF copies balanced DVE/ACT.
"""

import sys

sys.path.insert(0, "/opt/trn_rl_repo")

import numpy as np

import concourse.bass as bass
import concourse.tile as tile
from concourse import bacc, mybir
from concourse.bass_utils import run_bass_kernel_spmd


def _ensure_ntff_hook():
    """The agent image's antenv package lacks axon_hooks, which makes
    run_bass_kernel_spmd(trace=True) crash on import. Provide the module and
    register the ctypes-based NTFF profiling hook (degrades silently)."""
    try:
        import types

        import antenv

        if "antenv.axon_hooks" not in sys.modules:
            m = types.ModuleType("antenv.axon_hooks")
            state = {"hook": None}
            m.set_axon_ntff_profile_hook = lambda h: state.__setitem__("hook", h)
            m.get_axon_ntff_profile_hook = lambda: state["hook"]
            sys.modules["antenv.axon_hooks"] = m
            antenv.axon_hooks = m
        from antenv.axon_hooks import (
            get_axon_ntff_profile_hook,
            set_axon_ntff_profile_hook,
        )

        if get_axon_ntff_profile_hook() is None:
            from trn_agent_boot.trn_boot import _ntff_profile_via_ctypes

            set_axon_ntff_profile_hook(
                _ntff_profile_via_ctypes("/opt/axon/libaxon_pjrt.so")
            )
    except Exception:
        pass


_ensure_ntff_hook()

F32 = mybir.dt.float32
F16 = mybir.dt.float16
BF16 = mybir.dt.bfloat16

B, L, D, H = 2, 2048, 2048, 32
HD = 64          # head dim
N_CORES = 8
TP = 4           # tensor-parallel width (heads split 4 ways)
HPC = H // TP    # heads per core = 8
NHP = HPC // 2   # head pairs per core = 4
DH = HPC * HD    # per-core projected width = 512
SCALE = float(HD) ** -0.5

QB = 512         # query-block width for SDPA
KC = D // 128    # contraction chunks for the projections = 16
NT = L // QB     # 512-token tiles = 4
TC = L // 128    # 128-token chunks = 16
KPQ = QB // 128  # k-chunks per q-block = 4


def _emit(nc):
    xq = nc.dram_tensor("xq", [D, L], F16, kind="ExternalInput")
    xk = nc.dram_tensor("xk", [D, L], F16, kind="ExternalInput")
    xv = nc.dram_tensor("xv", [D, L], F16, kind="ExternalInput")
    wq = nc.dram_tensor("wq", [D, DH], F16, kind="ExternalInput")
    wk = nc.dram_tensor("wk", [D, DH], F16, kind="ExternalInput")
    wv = nc.dram_tensor("wv", [D, DH], F16, kind="ExternalInput")
    wo = nc.dram_tensor("wo", [DH, D], F16, kind="ExternalInput")
    # konst [128, 256]: cols 0:128 = triu ones (f16), cols 128:256 = identity
    konst = nc.dram_tensor("konst", [128, 256], F16, kind="ExternalInput")
    outT = nc.dram_tensor("outT", [D, L], F16, kind="ExternalOutput")

    EXP = mybir.ActivationFunctionType.Exp

    with tile.TileContext(nc) as tc:
        from contextlib import ExitStack

        with ExitStack() as st:
            constp = st.enter_context(tc.tile_pool(name="const", bufs=1))
            ksb = constp.tile([128, 256], F16)
            nc.sync.dma_start(ksb[:], konst[:])
            tri_sb = ksb[:, 0:128]          # triu ones, f16
            ident_sb = ksb[:, 128:256]      # identity, f16
            # small constants built on-device
            ones64 = constp.tile([128, 64], F16)
            nc.vector.memset(ones64[:], 1.0)
            tri_bf = constp.tile([128, 128], BF16)
            nc.vector.tensor_copy(tri_bf[:], tri_sb)

            actp = st.enter_context(tc.tile_pool(name="acts", bufs=1))
            qhT = actp.tile([128, NHP, L], F16)
            khT = actp.tile([128, NHP, L], F16)
            # vh: per 128-token chunk, 8 heads x (64 v-dims + ones col), bf16
            vh = actp.tile([128, TC, HPC * (HD + 1)], BF16)
            # set the whole tile to 1.0 once; projection copies overwrite the
            # data columns, leaving the 65th (denominator) column at 1.0
            nc.vector.memset(vh[:], 1.0)
            oT = actp.tile([128, NHP, L], F16)      # normalized per-head out
            oTB = actp.tile([64, NHP, L], F16)      # head-B staging (rows 0:64)
            # denominator rows, all on partition 64 (no partition shift):
            # [65, hp%2 slot, block-head-within-hp jl, 512]; two slots so the
            # deferred norm pass of hp can overlap hp+1's collection
            den_sb = actp.tile([65, 2, 8, QB], F16)

            # ---- q/k projections: out[dim_chunk, tokens] = w_chunk.T @ xT ----
            for name, xdram, wdram, dst in (("q", xq, wq, qhT), ("k", xk, wk, khT)):
                with (
                    tc.tile_pool(name=f"w{name}", bufs=1) as wp,
                    tc.tile_pool(name=f"x{name}", bufs=2) as xp,
                    tc.tile_pool(name=f"ps{name}", bufs=4, space="PSUM") as pp,
                ):
                    w_sb = wp.tile([128, KC, DH], F16, tag="w")
                    nc.sync.dma_start(
                        w_sb[:], wdram[:].rearrange("(kc p) m -> p kc m", p=128)
                    )
                    for n in range(NT):
                        x_sb = xp.tile([128, KC, QB], F16, tag="x")
                        nc.sync.dma_start(
                            x_sb[:],
                            xdram[:, n * QB : (n + 1) * QB].rearrange(
                                "(kc p) t -> p kc t", p=128
                            ),
                        )
                        for m in range(NHP):
                            ps = pp.tile([128, QB], F32, tag="ps")
                            for kc in range(KC):
                                nc.tensor.matmul(
                                    ps[:],
                                    w_sb[:, kc, m * 128 : (m + 1) * 128],
                                    x_sb[:, kc, :],
                                    start=(kc == 0),
                                    stop=(kc == KC - 1),
                                )
                            eng = nc.vector if (m % 2 == 0) else nc.scalar
                            if eng is nc.vector:
                                eng.tensor_copy(
                                    dst[:, m, n * QB : (n + 1) * QB], ps[:]
                                )
                            else:
                                eng.copy(dst[:, m, n * QB : (n + 1) * QB], ps[:])

            # ---- v projection: natural layout, x chunk is the stationary op ----
            with (
                tc.tile_pool(name="wvp", bufs=1) as wp,
                tc.tile_pool(name="xvp", bufs=2) as xp,
                tc.tile_pool(name="psv", bufs=4, space="PSUM") as pp,
            ):
                w_sb = wp.tile([128, KC, DH], F16, tag="w")
                nc.sync.dma_start(
                    w_sb[:], wv[:].rearrange("(kc p) m -> p kc m", p=128)
                )
                for n in range(NT):
                    x_sb = xp.tile([128, KC, QB], F16, tag="x")
                    nc.sync.dma_start(
                        x_sb[:],
                        xv[:, n * QB : (n + 1) * QB].rearrange(
                            "(kc p) t -> p kc t", p=128
                        ),
                    )
                    for tt in range(QB // 128):
                        ps = pp.tile([128, DH], F32, tag="ps")
                        for kc in range(KC):
                            nc.tensor.matmul(
                                ps[:],
                                x_sb[:, kc, tt * 128 : (tt + 1) * 128],
                                w_sb[:, kc, :],
                                start=(kc == 0),
                                stop=(kc == KC - 1),
                            )
                        tci = n * (QB // 128) + tt
                        vdst = vh[:, tci, :].rearrange("p (h c) -> p h c", c=HD + 1)
                        eng = nc.vector if (tt % 2 == 0) else nc.scalar
                        if eng is nc.vector:
                            eng.tensor_copy(
                                vdst[:, :, 0:HD],
                                ps[:].rearrange("p (h d) -> p h d", d=HD),
                            )
                        else:
                            eng.copy(
                                vdst[:, :, 0:HD],
                                ps[:].rearrange("p (h d) -> p h d", d=HD),
                            )

            # ---- SDPA ----
            # Blocks (hp, qb) processed qb-inner; within a block, a lag-2
            # software pipeline: S pair + exp for step i, PV pair for step
            # i-2.  Head A lives on partitions 0:64, head B on 64:128 of
            # qhT/khT (concurrent row-tiled S matmuls).
            with (
                tc.tile_pool(name="pp", bufs=8) as ppool,
                tc.tile_pool(name="sps", bufs=2, space="PSUM") as spool,
                tc.tile_pool(name="ops", bufs=3, space="PSUM") as opool,
                tc.tile_pool(name="dtp", bufs=1, space="PSUM") as dtpool,
                tc.tile_pool(name="rrp", bufs=4) as rrpool,
                tc.tile_pool(name="drp", bufs=2) as drpool,
            ):
                steps = []   # (hp, qb, kc) in emission order
                for hp in range(NHP):
                    for qb in range(NT):
                        for kc in range((qb + 1) * KPQ):
                            steps.append((hp, qb, kc))

                state = {}         # (hp, qb) -> dict with p tiles, o tiles
                pend_pv = []       # queued PV closures
                deferred = []      # (emit_after_idx, closure)
                LAG = 2

                def emit_s(i):
                    hp, qb, kc = steps[i]
                    key = (hp, qb)
                    if key not in state:
                        state[key] = {"p": {}, "o": None}
                    stt = state[key]
                    q0 = qb * QB
                    dj = kc - qb * KPQ
                    c0 = 128 * dj if dj > 0 else 0
                    spair = spool.tile([128, 2, QB], F32, tag="s", name="spair")
                    nc.tensor.matmul(
                        spair[:, 0, c0:QB],
                        khT[0:64, hp, kc * 128 : (kc + 1) * 128],
                        qhT[0:64, hp, q0 + c0 : q0 + QB],
                        start=True,
                        stop=True,
                    )
                    nc.tensor.matmul(
                        spair[:, 1, c0:QB],
                        khT[64:128, hp, kc * 128 : (kc + 1) * 128],
                        qhT[64:128, hp, q0 + c0 : q0 + QB],
                        start=True,
                        stop=True,
                    )
                    ppair = ppool.tile([128, 2, QB], BF16, tag="p", name="ppair")
                    nc.scalar.activation(
                        ppair[:, :, c0:QB], spair[:, :, c0:QB], EXP
                    )
                    if dj >= 0:
                        # causal triangle on the diagonal 128-strip (gpsimd)
                        for h in range(2):
                            nc.gpsimd.tensor_mul(
                                ppair[:, h, c0 : c0 + 128],
                                ppair[:, h, c0 : c0 + 128],
                                tri_bf[:],
                            )
                    stt["p"][kc] = ppair

                def emit_pv(i):
                    hp, qb, kc = steps[i]
                    key = (hp, qb)
                    stt = state[key]
                    kcnt = (qb + 1) * KPQ
                    dj = kc - qb * KPQ
                    c0 = 128 * dj if dj > 0 else 0
                    if kc == 0:
                        stt["o"] = (
                            opool.tile([65, QB], F32, tag="o", name="o_a"),
                            opool.tile([65, QB], F32, tag="o", name="o_b"),
                        )
                    ppair = stt["p"].pop(kc)
                    for h in range(2):
                        nc.tensor.matmul(
                            stt["o"][h][:, c0:QB],
                            vh[:, kc, (2 * hp + h) * (HD + 1) : (2 * hp + h + 1) * (HD + 1)],
                            ppair[:, h, c0:QB],
                            start=(kc == 0),
                            stop=(kc == kcnt - 1),
                        )
                    if kc == kcnt - 1:
                        close_block(hp, qb)

                def close_block(hp, qb):
                    # unnormalized o -> SBUF; denominator row -> den_sb
                    stt = state.pop((hp, qb))
                    q0 = qb * QB
                    oA, oB = stt["o"]
                    nc.vector.tensor_copy(oT[0:64, hp, q0 : q0 + QB], oA[0:64, :])
                    nc.vector.tensor_copy(oTB[0:64, hp, q0 : q0 + QB], oB[0:64, :])
                    nc.vector.tensor_copy(
                        den_sb[64:65, hp % 2, qb * 2, :], oA[64:65, :]
                    )
                    nc.vector.tensor_copy(
                        den_sb[64:65, hp % 2, qb * 2 + 1, :], oB[64:65, :]
                    )

                def norm_pass(hp):
                    # gather denominators onto partitions, one batched
                    # reciprocal, then PE-broadcast + normalize in place
                    def closure():
                        den_t = dtpool.tile([128, 32], F32, tag="dt", name="den_t")
                        for jl in range(8):          # block-head within hp
                            for c in range(4):       # 128-col chunk of 512
                                nc.tensor.matmul(
                                    den_t[:, jl * 4 + c : jl * 4 + c + 1],
                                    den_sb[
                                        64:65, hp % 2, jl, c * 128 : (c + 1) * 128
                                    ],
                                    ones64[64:65, 0:1],
                                    start=(jl == 0 and c == 0),
                                    stop=(jl == 7 and c == 3),
                                    skip_group_check=True,
                                )
                        den_rt = drpool.tile([128, 32], F32, tag="dr", name="den_rt")
                        nc.vector.reciprocal(den_rt[:], den_t[:])
                        for jl in range(8):
                            qb, h = jl // 2, jl % 2
                            q0 = qb * QB
                            bc = opool.tile([65, QB], F32, tag="o", name="bc")
                            for c in range(4):
                                rrep = rrpool.tile([128, 64], F16, tag="rr", name="rrep")
                                nc.vector.tensor_scalar_mul(
                                    rrep[:],
                                    ones64[:],
                                    den_rt[:, jl * 4 + c : jl * 4 + c + 1],
                                )
                                nc.tensor.matmul(
                                    bc[0:64, c * 128 : (c + 1) * 128],
                                    rrep[:],
                                    ident_sb[:],
                                    start=(c == 0),
                                    stop=(c == 3),
                                    skip_group_check=True,
                                )
                            dst = (
                                oT[0:64, hp, q0 : q0 + QB]
                                if h == 0
                                else oTB[0:64, hp, q0 : q0 + QB]
                            )
                            nc.vector.tensor_mul(dst, dst, bc[0:64, :])
                            if h == 1:
                                nc.sync.dma_start(
                                    oT[64:128, hp, q0 : q0 + QB],
                                    oTB[0:64, hp, q0 : q0 + QB],
                                )

                    return closure

                nsteps = len(steps)
                for i in range(nsteps + LAG):
                    if i < nsteps:
                        emit_s(i)
                    j = i - LAG
                    if j >= 0:
                        emit_pv(j)
                        hp, qb, kc = steps[j]
                        if qb == NT - 1 and kc == (qb + 1) * KPQ - 1:
                            # hp finished; defer its norm pass by ~one block
                            target = min(j + 4, nsteps - 1)
                            deferred.append((target, norm_pass(hp)))
                    # emit deferred norm passes once enough steps have passed
                    still = []
                    for tgt, clo in deferred:
                        if j >= tgt or i >= nsteps + LAG - 1:
                            clo()
                        else:
                            still.append((tgt, clo))
                    deferred = still

            # ---- output projection: outT[m,n] = wo_chunk.T @ oT ----
            with (
                tc.tile_pool(name="wop", bufs=1) as wop,
                tc.tile_pool(name="fps", bufs=8, space="PSUM") as fpool,
                tc.tile_pool(name="osbp", bufs=4) as osbp,
            ):
                wo_sb = wop.tile([128, NHP, D], F16)
                nc.sync.dma_start(
                    wo_sb[:], wo[:].rearrange("(kc p) m -> p kc m", p=128)
                )
                for m in range(D // 128):
                    pts = []
                    for n in range(NT):
                        pt = fpool.tile([128, QB], F32, tag="f", name="pt")
                        pts.append(pt)
                    for kc2 in range(NHP):
                        for n in range(NT):
                            nc.tensor.matmul(
                                pts[n][:],
                                wo_sb[:, kc2, m * 128 : (m + 1) * 128],
                                oT[:, kc2, n * QB : (n + 1) * QB],
                                start=(kc2 == 0),
                                stop=(kc2 == NHP - 1),
                            )
                    for n in range(NT):
                        osb = osbp.tile([128, QB], F16, tag="ot", name="osb")
                        if n % 2 == 0:
                            nc.vector.tensor_copy(osb[:], pts[n][:])
                        else:
                            nc.scalar.copy(osb[:], pts[n][:])
                        nc.sync.dma_start(
                            outT[m * 128 : (m + 1) * 128, n * QB : (n + 1) * QB],
                            osb[:],
                        )
    return nc


def build():
    nc = bacc.Bacc("TRN2", target_bir_lowering=False, debug=False)
    _emit(nc)
    nc.compile()
    return nc


_NC_CACHE = {}


def _get_nc():
    if "nc" not in _NC_CACHE:
        _NC_CACHE["nc"] = build()
    return _NC_CACHE["nc"]


def make_in_maps(q, k, v, Wq, Wk, Wv, Wo):
    konst_m = np.zeros((128, 256), dtype=np.float16)
    konst_m[:, 0:128] = np.triu(np.ones((128, 128), dtype=np.float16))
    konst_m[:, 128:256] = np.eye(128, dtype=np.float16)
    qT = [np.ascontiguousarray(q[b].T).astype(np.float16) for b in range(B)]
    kT = [np.ascontiguousarray(k[b].T).astype(np.float16) for b in range(B)]
    vT = [np.ascontiguousarray(v[b].T).astype(np.float16) for b in range(B)]
    wq_s, wk_s, wv_s, wo_s = [], [], [], []
    for tp in range(TP):
        rows = slice(tp * DH, (tp + 1) * DH)
        wq_s.append(np.ascontiguousarray(Wq[rows].T * SCALE).astype(np.float16))
        wk_s.append(np.ascontiguousarray(Wk[rows].T).astype(np.float16))
        wv_s.append(np.ascontiguousarray(Wv[rows].T).astype(np.float16))
        wo_s.append(np.ascontiguousarray(Wo[:, rows].T).astype(np.float16))
    in_maps = []
    for c in range(N_CORES):
        b, tp = c // TP, c % TP
        in_maps.append(
            {
                "xq": qT[b],
                "xk": kT[b],
                "xv": vT[b],
                "wq": wq_s[tp],
                "wk": wk_s[tp],
                "wv": wv_s[tp],
                "wo": wo_s[tp],
                "konst": konst_m,
            }
        )
    return in_maps


def kernel(q, k, v, Wq, Wk, Wv, Wo, mask=None, trace=False):
    q = np.asarray(q, dtype=np.float32)
    k = np.asarray(k, dtype=np.float32)
    v = np.asarray(v, dtype=np.float32)
    nc = _get_nc()
    in_maps = make_in_maps(
        q, k, v,
        np.asarray(Wq, np.float32), np.asarray(Wk, np.float32),
        np.asarray(Wv, np.float32), np.asarray(Wo, np.float32),
    )
    res = run_bass_kernel_spmd(
        nc, in_maps, core_ids=list(range(N_CORES)), trace=trace
    )
    out = np.zeros((B, L, D), dtype=np.float32)
    for c in range(N_CORES):
        out[c // TP] += res.results[c]["outT"].T.astype(np.float32)
    if trace:
        return out, res
    return out


# revision 9
# speedup vs baseline: 1.8527x; 1.1200x over previous
"""Causal multi-head attention (B=2, L=2048, D=2048, H=32) on 8 trn2 NeuronCores.

Sharding: data-parallel over batch (2 groups of 4 cores) x tensor-parallel over
heads (8 heads per core). Host pre-transposes x and pre-shards/scales weights;
host sums the 4 tensor-parallel partials per batch (fp32).

v3 design:
  - fp16 operands (bf16 for p/vh so exp of unmasked garbage can't overflow
    16-bit range); fp32 PSUM accumulation; fp16 output partials.
  - Fully interleaved token-tile pipeline: for each 512-token tile n, emit the
    q/k/v projections for tile n and then SDPA for q-block n (causality means
    its whole k-range is already projected).  ACT's exp stream overlaps the
    next tile's projection matmuls, so the PE never waits out the softmax.
  - x-tile DMAs emitted one section early (prefetch during SDPA).
  - S matmuls run as head PAIRS on row-tiles (0,0)/(64,0) (K=64 concurrent);
    exp batched over [128,2,512] PSUM double-tiles; PV restricted to the
    causally valid columns; lag-2 software pipeline S->exp->PV.
  - softmax denominators: per-block rows collected at partition 64, PE-gathered
    onto partitions, ONE batched reciprocal per q-section [128,32], PE
    broadcast back; normalization deferred one section (overlaps projections).
  - ACT is exp-only during the pipeline; projection PSUM copies + triangle
    masking on DVE; outproj copies split DVE/ACT.
"""

import sys

sys.path.insert(0, "/opt/trn_rl_repo")

import numpy as np

import concourse.bass as bass
import concourse.tile as tile
from concourse import bacc, mybir
from concourse.bass_utils import run_bass_kernel_spmd


def _ensure_ntff_hook():
    """The agent image's antenv package lacks axon_hooks, which makes
    run_bass_kernel_spmd(trace=True) crash on import. Provide the module and
    register the ctypes-based NTFF profiling hook (degrades silently)."""
    try:
        import types

        import antenv

        if "antenv.axon_hooks" not in sys.modules:
            m = types.ModuleType("antenv.axon_hooks")
            state = {"hook": None}
            m.set_axon_ntff_profile_hook = lambda h: state.__setitem__("hook", h)
            m.get_axon_ntff_profile_hook = lambda: state["hook"]
            sys.modules["antenv.axon_hooks"] = m
            antenv.axon_hooks = m
        from antenv.axon_hooks import (
            get_axon_ntff_profile_hook,
            set_axon_ntff_profile_hook,
        )

        if get_axon_ntff_profile_hook() is None:
            from trn_agent_boot.trn_boot import _ntff_profile_via_ctypes

            set_axon_ntff_profile_hook(
                _ntff_profile_via_ctypes("/opt/axon/libaxon_pjrt.so")
            )
    except Exception:
        pass


_ensure_ntff_hook()

F32 = mybir.dt.float32
F16 = mybir.dt.float16
BF16 = mybir.dt.bfloat16

B, L, D, H = 2, 2048, 2048, 32
HD = 64          # head dim
N_CORES = 8
TP = 4           # tensor-parallel width (heads split 4 ways)
HPC = H // TP    # heads per core = 8
NHP = HPC // 2   # head pairs per core = 4
DH = HPC * HD    # per-core projected width = 512
SCALE = float(HD) ** -0.5

QB = 512         # query-block width for SDPA
KC = D // 128    # contraction chunks for the projections = 16
NT = L // QB     # 512-token tiles = 4
TC = L // 128    # 128-token chunks = 16
KPQ = QB // 128  # k-chunks per q-block = 4


def _emit(nc):
    xq = nc.dram_tensor("xq", [D, L], F16, kind="ExternalInput")
    xk = nc.dram_tensor("xk", [D, L], F16, kind="ExternalInput")
    xv = nc.dram_tensor("xv", [D, L], F16, kind="ExternalInput")
    wq = nc.dram_tensor("wq", [D, DH], F16, kind="ExternalInput")
    wk = nc.dram_tensor("wk", [D, DH], F16, kind="ExternalInput")
    wv = nc.dram_tensor("wv", [D, DH], F16, kind="ExternalInput")
    wo = nc.dram_tensor("wo", [DH, D], F16, kind="ExternalInput")
    # konst [128, 256]: cols 0:128 = triu ones (f16), cols 128:256 = identity
    konst = nc.dram_tensor("konst", [128, 256], F16, kind="ExternalInput")
    outT = nc.dram_tensor("outT", [D, L], F16, kind="ExternalOutput")

    EXP = mybir.ActivationFunctionType.Exp

    with tile.TileContext(nc) as tc:
        from contextlib import ExitStack

        with ExitStack() as st:
            constp = st.enter_context(tc.tile_pool(name="const", bufs=1))
            ksb = constp.tile([128, 256], F16)
            nc.sync.dma_start(ksb[:], konst[:])
            ident_sb = ksb[:, 128:256]      # identity, f16
            ones64 = constp.tile([128, 64], F16)
            nc.vector.memset(ones64[:], 1.0)
            tri_bf = constp.tile([128, 128], BF16)
            nc.vector.tensor_copy(tri_bf[:], ksb[:, 0:128])

            actp = st.enter_context(tc.tile_pool(name="acts", bufs=1))
            qhT = actp.tile([128, NHP, L], F16)
            khT = actp.tile([128, NHP, L], F16)
            # vh: per 128-token chunk, 8 heads x (64 v-dims + ones col), bf16
            vh = actp.tile([128, TC, HPC * (HD + 1)], BF16)
            # whole tile starts at 1.0; projection copies overwrite the data
            # columns, leaving the 65th (denominator) column at 1.0
            nc.vector.memset(vh[:], 1.0)
            oT = actp.tile([128, NHP, L], F16)       # normalized per-head out
            oTB = actp.tile([64, 2, NHP, QB], F16)   # head-B unnorm, qb%2 slot
            # denominator rows at partition 64: [65, qb%2, jl=hp*2+h, 512]
            den_sb = actp.tile([65, 2, 8, QB], F16)

            # ---- persistent weights + per-section x tiles ----
            # (inner stack: closed before the output projection so its SBUF
            # is reused for wo/osb)
            ist = st.enter_context(ExitStack())
            wqp = ist.enter_context(tc.tile_pool(name="wqp", bufs=1))
            wkp = ist.enter_context(tc.tile_pool(name="wkp", bufs=1))
            wvp = ist.enter_context(tc.tile_pool(name="wvp", bufs=1))
            xqp = ist.enter_context(tc.tile_pool(name="xqp", bufs=1))
            xkp = ist.enter_context(tc.tile_pool(name="xkp", bufs=1))
            xvp = ist.enter_context(tc.tile_pool(name="xvp", bufs=1))
            wq_sb = wqp.tile([128, KC, DH], F16)
            wk_sb = wkp.tile([128, KC, DH], F16)
            wv_sb = wvp.tile([128, KC, DH], F16)
            for w_sb, wdram in ((wq_sb, wq), (wk_sb, wk), (wv_sb, wv)):
                nc.sync.dma_start(
                    w_sb[:], wdram[:].rearrange("(kc p) m -> p kc m", p=128)
                )

            ppool = ist.enter_context(tc.tile_pool(name="pp", bufs=6))
            projp = ist.enter_context(
                tc.tile_pool(name="projp", bufs=2, space="PSUM")
            )
            spool = ist.enter_context(
                tc.tile_pool(name="sps", bufs=2, space="PSUM")
            )
            opool = ist.enter_context(
                tc.tile_pool(name="ops", bufs=2, space="PSUM")
            )
            rrpool = ist.enter_context(tc.tile_pool(name="rrp", bufs=4))
            drpool = ist.enter_context(tc.tile_pool(name="drp", bufs=2))

            def emit_x_dma(n):
                tiles = {}
                for key, xp, xdram in (
                    ("q", xqp, xq), ("k", xkp, xk), ("v", xvp, xv)
                ):
                    x_sb = xp.tile([128, KC, QB], F16, tag="x", name=f"x{key}")
                    nc.sync.dma_start(
                        x_sb[:],
                        xdram[:, n * QB : (n + 1) * QB].rearrange(
                            "(kc p) t -> p kc t", p=128
                        ),
                    )
                    tiles[key] = x_sb
                return tiles

            def emit_proj(n, xt):
                # q/k projections: head dims on partitions
                for w_sb, dst, key in (
                    (wq_sb, qhT, "q"), (wk_sb, khT, "k")
                ):
                    for m in range(NHP):
                        ps = projp.tile([128, QB], F32, tag="ps", name="ps")
                        for kc in range(KC):
                            nc.tensor.matmul(
                                ps[:],
                                w_sb[:, kc, m * 128 : (m + 1) * 128],
                                xt[key][:, kc, :],
                                start=(kc == 0),
                                stop=(kc == KC - 1),
                            )
                        nc.vector.tensor_copy(
                            dst[:, m, n * QB : (n + 1) * QB], ps[:]
                        )
                # v projection: tokens on partitions
                for tt in range(KPQ):
                    ps = projp.tile([128, QB], F32, tag="ps", name="ps")
                    for kc in range(KC):
                        nc.tensor.matmul(
                            ps[:, 0:DH],
                            xt["v"][:, kc, tt * 128 : (tt + 1) * 128],
                            wv_sb[:, kc, :],
                            start=(kc == 0),
                            stop=(kc == KC - 1),
                        )
                    tci = n * KPQ + tt
                    vdst = vh[:, tci, :].rearrange("p (h c) -> p h c", c=HD + 1)
                    nc.vector.tensor_copy(
                        vdst[:, :, 0:HD],
                        ps[:, 0:DH].rearrange("p (h d) -> p h d", d=HD),
                    )

            # ---- SDPA section for q-block qb: blocks (hp, qb), lag-2 ----
            def emit_sdpa_section(qb):
                kcnt = (qb + 1) * KPQ
                q0 = qb * QB
                steps = [(hp, kc) for hp in range(NHP) for kc in range(kcnt)]
                state = {}

                def emit_s(i):
                    hp, kc = steps[i]
                    if hp not in state:
                        state[hp] = {"p": {}, "o": None}
                    stt = state[hp]
                    dj = kc - qb * KPQ
                    c0 = 128 * dj if dj > 0 else 0
                    spair = spool.tile([128, 2, QB], F32, tag="s", name="spair")
                    nc.tensor.matmul(
                        spair[:, 0, c0:QB],
                        khT[0:64, hp, kc * 128 : (kc + 1) * 128],
                        qhT[0:64, hp, q0 + c0 : q0 + QB],
                        start=True,
                        stop=True,
                    )
                    nc.tensor.matmul(
                        spair[:, 1, c0:QB],
                        khT[64:128, hp, kc * 128 : (kc + 1) * 128],
                        qhT[64:128, hp, q0 + c0 : q0 + QB],
                        start=True,
                        stop=True,
                    )
                    ppair = ppool.tile([128, 2, QB], BF16, tag="p", name="ppair")
                    nc.scalar.activation(
                        ppair[:, :, c0:QB], spair[:, :, c0:QB], EXP
                    )
                    if dj >= 0:
                        for h in range(2):
                            nc.vector.tensor_mul(
                                ppair[:, h, c0 : c0 + 128],
                                ppair[:, h, c0 : c0 + 128],
                                tri_bf[:],
                            )
                    stt["p"][kc] = ppair

                def emit_pv(i):
                    hp, kc = steps[i]
                    stt = state[hp]
                    dj = kc - qb * KPQ
                    c0 = 128 * dj if dj > 0 else 0
                    if kc == 0:
                        stt["o"] = (
                            opool.tile([65, QB], F32, tag="o", name="o_a"),
                            opool.tile([65, QB], F32, tag="o", name="o_b"),
                        )
                    ppair = stt["p"].pop(kc)
                    for h in range(2):
                        nc.tensor.matmul(
                            stt["o"][h][:, c0:QB],
                            vh[:, kc, (2 * hp + h) * (HD + 1) : (2 * hp + h + 1) * (HD + 1)],
                            ppair[:, h, c0:QB],
                            start=(kc == 0),
                            stop=(kc == kcnt - 1),
                        )
                    if kc == kcnt - 1:
                        # unnormalized o -> SBUF; denominator rows -> den_sb
                        stt = state.pop(hp)
                        oA, oB = stt["o"]
                        nc.vector.tensor_copy(
                            oT[0:64, hp, q0 : q0 + QB], oA[0:64, :]
                        )
                        nc.vector.tensor_copy(
                            oTB[0:64, qb % 2, hp, :], oB[0:64, :]
                        )
                        nc.vector.tensor_copy(
                            den_sb[64:65, qb % 2, hp * 2, :], oA[64:65, :]
                        )
                        nc.vector.tensor_copy(
                            den_sb[64:65, qb % 2, hp * 2 + 1, :], oB[64:65, :]
                        )

                LAG = 2
                for i in range(len(steps) + LAG):
                    if i < len(steps):
                        emit_s(i)
                    if i - LAG >= 0:
                        emit_pv(i - LAG)

            # ---- deferred normalization for section qb ----
            def emit_norm(qb):
                q0 = qb * QB
                den_t = projp.tile([128, QB], F32, tag="ps", name="den_t")
                for jl in range(8):              # jl = hp*2 + head
                    for c in range(4):
                        nc.tensor.matmul(
                            den_t[:, jl * 4 + c : jl * 4 + c + 1],
                            den_sb[64:65, qb % 2, jl, c * 128 : (c + 1) * 128],
                            ones64[64:65, 0:1],
                            start=(jl == 0 and c == 0),
                            stop=(jl == 7 and c == 3),
                            skip_group_check=True,
                        )
                den_rt = drpool.tile([128, 32], F32, tag="dr", name="den_rt")
                nc.vector.reciprocal(den_rt[:], den_t[:, 0:32])
                for jl in range(8):
                    hp, h = jl // 2, jl % 2
                    bc = projp.tile([128, QB], F32, tag="ps", name="bc")
                    for c in range(4):
                        rrep = rrpool.tile([128, 64], F16, tag="rr", name="rrep")
                        nc.vector.tensor_scalar_mul(
                            rrep[:],
                            ones64[:],
                            den_rt[:, jl * 4 + c : jl * 4 + c + 1],
                        )
                        nc.tensor.matmul(
                            bc[0:64, c * 128 : (c + 1) * 128],
                            rrep[:],
                            ident_sb[:],
                            start=(c == 0),
                            stop=(c == 3),
                            skip_group_check=True,
                        )
                    if h == 0:
                        dst = oT[0:64, hp, q0 : q0 + QB]
                        nc.vector.tensor_mul(dst, dst, bc[0:64, :])
                    else:
                        dst = oTB[0:64, qb % 2, hp, :]
                        nc.vector.tensor_mul(dst, dst, bc[0:64, :])
                        nc.sync.dma_start(
                            oT[64:128, hp, q0 : q0 + QB], dst
                        )

            # ---- main interleaved pipeline ----
            xt = emit_x_dma(0)
            for n in range(NT):
                emit_proj(n, xt)
                if n + 1 < NT:
                    xt = emit_x_dma(n + 1)
                if n >= 1:
                    emit_norm(n - 1)
                emit_sdpa_section(n)
            emit_norm(NT - 1)
            ist.close()

            # ---- output projection: outT[m,n] = wo_chunk.T @ oT ----
            with (
                tc.tile_pool(name="wop", bufs=1) as wop,
                tc.tile_pool(name="fps", bufs=8, space="PSUM") as fpool,
                tc.tile_pool(name="osbp", bufs=4) as osbp,
            ):
                wo_sb = wop.tile([128, NHP, D], F16)
                nc.sync.dma_start(
                    wo_sb[:], wo[:].rearrange("(kc p) m -> p kc m", p=128)
                )
                for m in range(D // 128):
                    pts = []
                    for n in range(NT):
                        pt = fpool.tile([128, QB], F32, tag="f", name="pt")
                        pts.append(pt)
                    for kc2 in range(NHP):
                        for n in range(NT):
                            nc.tensor.matmul(
                                pts[n][:],
                                wo_sb[:, kc2, m * 128 : (m + 1) * 128],
                                oT[:, kc2, n * QB : (n + 1) * QB],
                                start=(kc2 == 0),
                                stop=(kc2 == NHP - 1),
                            )
                    for n in range(NT):
                        osb = osbp.tile([128, QB], F16, tag="ot", name="osb")
                        if n % 2 == 0:
                            nc.vector.tensor_copy(osb[:], pts[n][:])
                        else:
                            nc.scalar.copy(osb[:], pts[n][:])
                        nc.sync.dma_start(
                            outT[m * 128 : (m + 1) * 128, n * QB : (n + 1) * QB],
                            osb[:],
                        )
    return nc


def build():
    nc = bacc.Bacc("TRN2", target_bir_lowering=False, debug=False)
    _emit(nc)
    nc.compile()
    return nc


_NC_CACHE = {}


def _get_nc():
    if "nc" not in _NC_CACHE:
        _NC_CACHE["nc"] = build()
    return _NC_CACHE["nc"]


def make_in_maps(q, k, v, Wq, Wk, Wv, Wo):
    konst_m = np.zeros((128, 256), dtype=np.float16)
    konst_m[:, 0:128] = np.triu(np.ones((128, 128), dtype=np.float16))
    konst_m[:, 128:256] = np.eye(128, dtype=np.float16)
    qT = [np.ascontiguousarray(q[b].T).astype(np.float16) for b in range(B)]
    kT = [np.ascontiguousarray(k[b].T).astype(np.float16) for b in range(B)]
    vT = [np.ascontiguousarray(v[b].T).astype(np.float16) for b in range(B)]
    wq_s, wk_s, wv_s, wo_s = [], [], [], []
    for tp in range(TP):
        rows = slice(tp * DH, (tp + 1) * DH)
        wq_s.append(np.ascontiguousarray(Wq[rows].T * SCALE).astype(np.float16))
        wk_s.append(np.ascontiguousarray(Wk[rows].T).astype(np.float16))
        wv_s.append(np.ascontiguousarray(Wv[rows].T).astype(np.float16))
        wo_s.append(np.ascontiguousarray(Wo[:, rows].T).astype(np.float16))
    in_maps = []
    for c in range(N_CORES):
        b, tp = c // TP, c % TP
        in_maps.append(
            {
                "xq": qT[b],
                "xk": kT[b],
                "xv": vT[b],
                "wq": wq_s[tp],
                "wk": wk_s[tp],
                "wv": wv_s[tp],
                "wo": wo_s[tp],
                "konst": konst_m,
            }
        )
    return in_maps


def kernel(q, k, v, Wq, Wk, Wv, Wo, mask=None, trace=False):
    q = np.asarray(q, dtype=np.float32)
    k = np.asarray(k, dtype=np.float32)
    v = np.asarray(v, dtype=np.float32)
    nc = _get_nc()
    in_maps = make_in_maps(
        q, k, v,
        np.asarray(Wq, np.float32), np.asarray(Wk, np.float32),
        np.asarray(Wv, np.float32), np.asarray(Wo, np.float32),
    )
    res = run_bass_kernel_spmd(
        nc, in_maps, core_ids=list(range(N_CORES)), trace=trace
    )
    out = np.zeros((B, L, D), dtype=np.float32)
    for c in range(N_CORES):
        out[c // TP] += res.results[c]["outT"].T.astype(np.float32)
    if trace:
        return out, res
    return out
